# revision 1
# baseline (speedup 1.0000x reference)
"""Trainium2 Bass kernel v2 for nn_CRAP_16544214024675 (sparse_attention).

Reference computation (per batch b, channel c):
  q = Wq@feat + bq                        (1x1 conv over channels)
  k = unfold3x3_s2(src)                   (strided window gather, pad 1)
  v = unfold3x3_s2(Wv@src + bv)
  A = softmax_t( sum_px q*k_t / 64 )      (9 window positions)
  out = fold3x3_s1( A_t * v_t ) * feat

Sharding: 8 cores = 4 batches x 2 output-channel halves (slot 0 of packed
inputs = the core's own half; channel sums commute).

v2 vs v1 (80.8us -> 79.6us, rel err 6.5e-3):
 - DMA: no f32 featf (final multiply uses bf16 featb slot 0), output bf16
   (host upcasts): 14.9MB -> 11.2MB round-trip per core.
 - all four planes conv+bias-copy (ACT), folds batched late as plain
   diag(exp_t) matmuls into a zero-prefilled full-image fold PSUM
   (prefill makes accumulation order free; has_written handled).
 - all 9 logit products DVE 2x tensor_mul (odd-column t's read a
   pre-shifted q copy to stay 4B-aligned); reduce per t on ACT
   (Copy+accum) or fused into a 1x STT per REDUCE_MAP (tail t = STT).
 - longer PE warm-up + dependency-free filler bursts between plane convs
   to hold the HAM clock gate open across DMA gaps.
"""
import sys
from contextlib import ExitStack

import numpy as np

for _p in ("/opt/trn_rl_repo", "/root/.axon_site/_ro/trn_rl_repo"):
    if _p not in sys.path:
        sys.path.append(_p)

import ml_dtypes

import concourse.tile as tile
from concourse import bacc, mybir
from concourse import bass_utils
from concourse.bass_interp import get_hw_module

F32 = mybir.dt.float32
BF16 = mybir.dt.bfloat16
AF = mybir.ActivationFunctionType
ALU = mybir.AluOpType

B, C, H, W = 4, 256, 64, 64
N_CORES = 8

# plane (p,q) DMA/processing order; plane (1,1) first (its 4 t's pipeline
# early), single-t plane (0,0) last (shortest possible tail).
PLANE_ORDER = [(1, 1), (1, 0), (0, 1), (0, 0)]
PLANE_TS = {
    (1, 1): [(2, 2), (0, 0), (0, 2), (2, 0)],
    (1, 0): [(0, 1), (2, 1)],
    (0, 1): [(1, 0), (1, 2)],
    (0, 0): [(1, 1)],
}
COPY_PLANES = {(1, 1), (1, 0), (0, 1), (0, 0)}  # all planes conv+copy
T_ORDER = [t for pl in PLANE_ORDER for t in PLANE_TS[pl]]
T_IDX = {t: i for i, t in enumerate(T_ORDER)}
FUSED_T = [t for pl in PLANE_ORDER if pl not in COPY_PLANES
           for t in PLANE_TS[pl]]
# reduce engine per t: 'A' = ACT Copy+accum after the 2x product,
# 'S' = DVE fused scalar_tensor_tensor (1x, no ACT)
REDUCE_MAP = {
    (2, 2): 'A', (0, 0): 'S', (0, 2): 'S', (2, 0): 'S',
    (0, 1): 'A', (2, 1): 'A',
    (1, 0): 'S', (1, 2): 'A',
    (1, 1): 'S',
}
N_WARM = 52


def _windows(i, j):
    if i == 0:
        yo0, yo1, dy = 0, 63, 0
    elif i == 1:
        yo0, yo1, dy = 0, 64, 0
    else:
        yo0, yo1, dy = 1, 64, -1
    if j == 0:
        xo0, xo1, dx = 0, 63, 0
    elif j == 1:
        xo0, xo1, dx = 0, 64, 0
    else:
        xo0, xo1, dx = 1, 64, -1
    return yo0, yo1, dy, xo0, xo1, dx


def build_program():
    nc = bacc.Bacc("TRN2", target_bir_lowering=False, debug=False)

    featb_d = nc.dram_tensor("featb", (2, 128, H, W), BF16, kind="ExternalInput")
    spl_d = nc.dram_tensor("splanes", (4, 2, 128, H, W), BF16, kind="ExternalInput")
    wq_d = nc.dram_tensor("wq", (2, 128, 128), BF16, kind="ExternalInput")
    wv_d = nc.dram_tensor("wv", (2, 128, 128), BF16, kind="ExternalInput")
    bq_d = nc.dram_tensor("bq", (128, 1), F32, kind="ExternalInput")
    bv_d = nc.dram_tensor("bv", (128, 1), F32, kind="ExternalInput")
    id_d = nc.dram_tensor("identb", (128, 128), BF16, kind="ExternalInput")
    out_d = nc.dram_tensor("out", (128, H, W), BF16, kind="ExternalOutput")

    with tile.TileContext(nc) as tc, ExitStack() as ctx:
        pool = ctx.enter_context(tc.tile_pool(name="main", bufs=1))
        scpool = ctx.enter_context(tc.tile_pool(name="scratch", bufs=6))
        wtpool = ctx.enter_context(tc.tile_pool(name="wts", bufs=9))

        # --- input DMA on sync HWDGE, consumer order ---
        wq_t = pool.tile([128, 2, 128], BF16, tag="wq")
        wv_t = pool.tile([128, 2, 128], BF16, tag="wv")
        bq_t = pool.tile([128, 1], F32, tag="bq")
        bv_t = pool.tile([128, 1], F32, tag="bv")
        nc.sync.dma_start(wq_t[:], wq_d.ap().rearrange("a p b -> p a b"))
        nc.sync.dma_start(bq_t[:], bq_d.ap())
        featb_t = []
        for k in range(2):
            t_ = pool.tile([128, H, W], BF16, tag=f"featb{k}")
            nc.sync.dma_start(t_[:], featb_d.ap()[k])
            featb_t.append(t_)
        nc.sync.dma_start(wv_t[:], wv_d.ap().rearrange("a p b -> p a b"))
        nc.sync.dma_start(bv_t[:], bv_d.ap())
        id_t = pool.tile([128, 128], BF16, tag="identb")
        nc.sync.dma_start(id_t[:], id_d.ap())
        splane = {}
        for pi, (p, q) in enumerate(PLANE_ORDER):
            for ct in range(2):
                t_ = pool.tile([128, H, W], BF16, tag=f"spl{ct}{p}{q}",
                               name=f"spl{ct}{p}{q}")
                nc.sync.dma_start(t_[:], spl_d.ap()[pi, ct])
                splane[(ct, p, q)] = t_

        lg_t = pool.tile([128, 16], F32, tag="lg")
        exp_t = pool.tile([128, 16], F32, tag="exp")
        q_t = pool.tile([128, H, W], BF16, tag="q")
        qs_t = pool.tile([128, H * W], BF16, tag="qs")
        zero_t = pool.tile([128, 128], BF16, tag="zero")
        nc.gpsimd.memset(zero_t[:], 0.0)

        def conv_chunk(ps, w_t, srcs, half):
            r0 = 32 * half
            for k in range(2):
                for s in range(4):
                    nc.tensor.matmul(
                        ps[:, 8 * s: 8 * s + 8, :],
                        w_t[:, k, :],
                        srcs[k][:, r0 + 8 * s: r0 + 8 * s + 8, :],
                        start=(k == 0),
                        stop=(k == 1),
                    )

        with tc.tile_pool(name="psq", bufs=2, space="PSUM") as psq:
            # PE warm-up burst (dependency-free) to open the HAM clock gate
            warm_t = pool.tile([128, 256], BF16, tag="warm")
            nc.gpsimd.memset(warm_t[:], 0.5)
            wps = psq.tile([128, 32, W], F32, tag="ps", name="warmps")
            for w_i in range(N_WARM):
                nc.tensor.matmul(
                    wps[:, 0:2, :], warm_t[:, 0:128], warm_t[:, 128:256],
                    start=True, stop=True, skip_group_check=True,
                )

            # q-conv (bias fused on the ACT copy)
            for half in range(2):
                ps = psq.tile([128, 32, W], F32, tag="ps")
                conv_chunk(ps, wq_t, featb_t, half)
                with tc.high_priority():
                    nc.scalar.activation(q_t[:, 32 * half:32 * half + 32, :],
                                         ps[:], AF.Identity, bias=bq_t[:])

            # v-convs of all four planes (copies on ACT; DVE must stay
            # free for the product chain)
            vplane = {}
            for (p, q) in PLANE_ORDER:
                vp = pool.tile([128, H, W], BF16, tag=f"vpl{p}{q}",
                               name=f"vpl{p}{q}")
                vplane[(p, q)] = vp
                for half in range(2):
                    ps = psq.tile([128, 32, W], F32, tag="ps")
                    conv_chunk(ps, wv_t,
                               [splane[(0, p, q)], splane[(1, p, q)]], half)
                    with tc.high_priority():
                        nc.scalar.activation(
                            vp[:, 32 * half:32 * half + 32, :], ps[:],
                            AF.Identity, bias=bv_t[:])
                # dependency-free filler burst: keep the HAM clock gate open
                # across the DMA gap until the next plane lands
                fps = psq.tile([128, 32, W], F32, tag="ps", name=f"fill{p}{q}")
                for _f in range(18):
                    nc.tensor.matmul(
                        fps[:, 0:2, :], warm_t[:, 0:128], warm_t[:, 128:256],
                        start=True, stop=True, skip_group_check=True,
                    )

        # shifted q copy: qs_flat[i] = q_flat[i+1] (keeps the odd-column t's
        # 4B-aligned for DVE 2x). Row-end elements are never read.
        qf = q_t[:].rearrange("p h w -> p (h w)")
        nc.vector.tensor_copy(qs_t[:, 0:H * W - 1], qf[:, 1:H * W])

        # --- per-t: product (DVE 2x) -> reduce (ACT or STT) -> exp ->
        #     diag build (DVE).  Folds run later, batched. ---
        qs3 = qs_t[:].rearrange("p (h w) -> p h w", h=H)
        diags = {}
        for (p, q) in PLANE_ORDER:
            pl = splane[(0, p, q)]
            for (i, j) in PLANE_TS[(p, q)]:
                idx = T_IDX[(i, j)]
                if i == 0:
                    qr0, pr0, rows = 1, 0, 63
                else:
                    qr0, pr0, rows = 0, 0, 64
                if j == 0:
                    qc0, pc0, cols = 1, 0, 63
                else:
                    qc0, pc0, cols = 0, 0, 64
                if j == 0:
                    q_view = qs3[:, qr0:qr0 + rows, 0:cols]
                else:
                    q_view = q_t[:, qr0:qr0 + rows, qc0:qc0 + cols]
                p_view = pl[:, pr0:pr0 + rows, pc0:pc0 + cols]
                sc = scpool.tile([128, H, W], BF16, tag="prod",
                                 name=f"prod{idx}")
                if REDUCE_MAP[(i, j)] == 'A':
                    with tc.high_priority():
                        nc.vector.tensor_mul(sc[:, 0:rows, 0:cols],
                                             q_view, p_view)
                    nc.scalar.activation(
                        sc[:, 0:rows, 0:cols], sc[:, 0:rows, 0:cols],
                        AF.Copy, accum_out=lg_t[:, idx:idx + 1])
                else:
                    with tc.high_priority():
                        nc.vector.scalar_tensor_tensor(
                            out=sc[:, 0:rows, 0:cols], in0=q_view,
                            scalar=1.0, in1=p_view,
                            op0=ALU.mult, op1=ALU.mult,
                            accum_out=lg_t[:, idx:idx + 1])
                with tc.high_priority():
                    nc.scalar.activation(
                        exp_t[:, idx:idx + 1], lg_t[:, idx:idx + 1],
                        AF.Exp, scale=1.0 / 64.0)
                dg = wtpool.tile([128, 128], BF16, tag="wt",
                                 name=f"dg{idx}")
                diags[(i, j)] = dg
                with tc.high_priority():
                    nc.vector.tensor_scalar(
                        out=dg[:], in0=id_t[:],
                        scalar1=exp_t[:, idx:idx + 1], scalar2=None,
                        op0=ALU.mult)

        # --- fold phase: zero-prefill then 9 diag matmul passes ---
        psf = ctx.enter_context(tc.tile_pool(name="psf", bufs=1, space="PSUM"))
        fold_ps = psf.tile([128, H, W], F32, tag="fold")
        for s in range(8):
            nc.tensor.matmul(
                fold_ps[:, 8 * s:8 * s + 8, :], zero_t[:],
                featb_t[0][:, 8 * s:8 * s + 8, :],
                start=True, stop=False, skip_group_check=True,
            )
        last_t = T_ORDER[-1]
        for (i, j) in T_ORDER:
            yo0, yo1, dy, xo0, xo1, dx = _windows(i, j)
            pq = ((i + 1) % 2, (j + 1) % 2)
            vp = vplane[pq]
            dg = diags[(i, j)]
            stop = (i, j) == last_t
            yb = yo0
            while yb < yo1:
                ye = min((yb // 8 + 1) * 8, yo1)
                nc.tensor.matmul(
                    fold_ps[:, yb:ye, xo0:xo1],
                    dg[:],
                    vp[:, yb + dy:ye + dy, xo0 + dx:xo1 + dx],
                    start=False, stop=stop,
                    skip_group_check=True,
                )
                yb = ye

        # --- normalization + final: out = (fold * rz) * feat ---
        z_t = pool.tile([128, 8], F32, tag="z")
        rz_t = pool.tile([128, 1], F32, tag="rz")
        nc.vector.tensor_reduce(z_t[:, 0:1], exp_t[:, 0:9],
                                axis=mybir.AxisListType.X, op=ALU.add)
        nc.vector.reciprocal(rz_t[:], z_t[:, 0:1])
        out_t = pool.tile([128, H, W], BF16, tag="out")
        sc_f = pool.tile([128, 32, W], BF16, tag="scf")
        sc_f2 = pool.tile([128, 32, W], BF16, tag="scf2")
        for half in range(2):
            y0 = 32 * half
            scf = sc_f if half == 0 else sc_f2
            with tc.high_priority():
                nc.scalar.activation(scf[:], fold_ps[:, y0:y0 + 32, :],
                                     AF.Identity, scale=rz_t[:])
                nc.vector.tensor_mul(out_t[:, y0:y0 + 32, :], scf[:],
                                     featb_t[0][:, y0:y0 + 32, :])
            nc.sync.dma_start(out_d.ap()[:, y0:y0 + 32, :],
                              out_t[:, y0:y0 + 32, :])

    nc.compile()
    nc.m = get_hw_module(nc.m)
    return nc


_PROGRAM = None


def _get_program():
    global _PROGRAM
    if _PROGRAM is None:
        _PROGRAM = build_program()
    return _PROGRAM


def _prep_inputs(feat, src, Wq, bq, Wv, bv):
    bf = ml_dtypes.bfloat16
    # src parity planes: (B, ct, p, q, 128, H, W)
    spl = np.ascontiguousarray(
        src.reshape(B, 2, 128, H, 2, W, 2).transpose(0, 1, 4, 6, 2, 3, 5)
    ).astype(bf)
    featb = feat.reshape(B, 2, 128, H, W).astype(bf)
    wq3 = np.ascontiguousarray(Wq.T).reshape(2, 128, C)
    wv3 = np.ascontiguousarray(Wv.T).reshape(2, 128, C)
    identb = np.eye(128, dtype=np.float32).astype(bf)
    in_maps = []
    for core in range(N_CORES):
        b, h = divmod(core, 2)
        oc = slice(h * 128, h * 128 + 128)
        order = [h, 1 - h]  # slot 0 = own input-channel half
        spl_core = np.stack([
            np.stack([spl[b][order[0]][p][q], spl[b][order[1]][p][q]])
            for (p, q) in PLANE_ORDER
        ])
        in_maps.append(
            dict(
                featb=np.ascontiguousarray(featb[b][order]),
                splanes=np.ascontiguousarray(spl_core),
                wq=np.ascontiguousarray(wq3[order][:, :, oc]).astype(bf),
                wv=np.ascontiguousarray(wv3[order][:, :, oc]).astype(bf),
                bq=bq[oc].reshape(128, 1).astype(np.float32),
                bv=bv[oc].reshape(128, 1).astype(np.float32),
                identb=identb,
            )
        )
    return in_maps


def kernel(feat, src, Wq, bq, Wv, bv, _trace=False):
    feat = np.asarray(feat, np.float32)
    src = np.asarray(src, np.float32)
    Wq = np.asarray(Wq, np.float32)
    bq = np.asarray(bq, np.float32)
    Wv = np.asarray(Wv, np.float32)
    bv = np.asarray(bv, np.float32)

    in_maps = _prep_inputs(feat, src, Wq, bq, Wv, bv)
    nc = _get_program()
    res = bass_utils.run_bass_kernel_spmd(
        nc, in_maps, core_ids=list(range(N_CORES)), trace=_trace
    )
    out = np.empty((B, C, H, W), np.float32)
    for core in range(N_CORES):
        b, h = divmod(core, 2)
        out[b, h * 128:h * 128 + 128] = np.asarray(
            res.results[core]["out"], dtype=np.float32)
    if _trace:
        kernel.last_results = res
    return out


kernel.last_results = None

